# revision 4
# baseline (speedup 1.0000x reference)
"""Trainium2 Bass kernel for the GNN edge-MLP (nn_BMA_update):

    out[e] = relu( relu([x[src]|x[dst]] @ W_nnn + b_nnn)
                 + relu(edge_attr @ W_root + b_root) ) @ W_out -> relu

Strategy (edge-parallel across 8 NeuronCores, bf16 data movement):
  - Host shards edges into 8 contiguous slices; within each slice, edges are
    bucketed by (src_bank, dst_bank) over 4 banks of 25000 node rows so the
    GPSIMD dma_gather ucode (int16 indices) can fetch node rows; buckets are
    padded to a fixed per-bucket capacity so the device program is
    input-independent. The host un-permutes the per-edge output at the end.
  - Node features are staged in two bf16 tables padded to 128 channels:
    xsrc[i] = [x[i] | 0], xdst[i] = [0 | x[i]].  Two bulk non-transpose
    dma_gathers per bucket (multi-queue, 16 SDMA engines) fetch the rows; a
    single DVE add then builds pair rows [x[src] | x[dst]] with no staging
    copies, and PE transposes (static offsets; loop fully unrolled) flip each
    [128e x 128ch] block to matmul orientation.
  - edge_attr is pre-transposed on the host per bucket ([64ch, cap] bf16), so
    h2's matmul rhs loads directly with plain contiguous DMA.
  - Per 512-edge megatile: three bf16 matmuls stream edges through
    W_nnn/W_root/W_out (PSUM f32), ACT fuses bias+relu for h1 and the output,
    DVE does bias+relu for h2 and the h1+h2 add.  The output stays transposed
    ([128 out_ch, E_pad] bf16 in DRAM); the host transposes back to [E, 128]
    f32 (blocked, cheap) during un-permutation.
"""
import numpy as np
import ml_dtypes

import concourse.bacc as bacc
import concourse.mybir as mybir
import concourse.tile as tile
from concourse import bass_utils
from concourse.masks import make_identity

N_NODES = 100000
N_EDGES = 1600000
NODE_C = 64
EDGE_C = 64
HIDDEN_C = 128
OUT_C = 128
N_CORES = 8
N_BANKS = 4
BANK = N_NODES // N_BANKS          # 25000 (< 32768 so int16 indices work)
N_BUCKETS = N_BANKS * N_BANKS      # 16
MEGA = 512                         # edges per megatile
DEFAULT_CAP = 14336                # 28 megatiles; mean bucket load 12500
F32, BF16, I16 = mybir.dt.float32, mybir.dt.bfloat16, mybir.dt.int16
BF16_NP = ml_dtypes.bfloat16

_BUILD_CACHE = {}


def _build_kernel(cap, n_queues=4, repeat=1):
    key = (cap, n_queues, repeat)
    if key in _BUILD_CACHE:
        return _BUILD_CACHE[key]
    n_mega = cap // MEGA
    E_pad = N_BUCKETS * cap
    S = cap // 16

    nc = bacc.Bacc("TRN2", target_bir_lowering=False, debug=False,
                   num_swdge_queues=n_queues)
    xsrc = nc.dram_tensor("xsrc", (N_NODES, 128), BF16, kind="ExternalInput")
    xdst = nc.dram_tensor("xdst", (N_NODES, 128), BF16, kind="ExternalInput")
    sidx = nc.dram_tensor("sidx", (N_BUCKETS, 128, S), I16, kind="ExternalInput")
    didx = nc.dram_tensor("didx", (N_BUCKETS, 128, S), I16, kind="ExternalInput")
    attrt = nc.dram_tensor("attrt", (N_BUCKETS, EDGE_C, cap), BF16,
                           kind="ExternalInput")
    Wnnn = nc.dram_tensor("Wnnn", (2 * NODE_C, HIDDEN_C), BF16, kind="ExternalInput")
    Wroot = nc.dram_tensor("Wroot", (EDGE_C, HIDDEN_C), BF16, kind="ExternalInput")
    Wout = nc.dram_tensor("Wout", (HIDDEN_C, OUT_C), BF16, kind="ExternalInput")
    bnnn = nc.dram_tensor("bnnn", (HIDDEN_C, 1), F32, kind="ExternalInput")
    broot = nc.dram_tensor("broot", (HIDDEN_C, 1), F32, kind="ExternalInput")
    bout = nc.dram_tensor("bout", (OUT_C, 1), F32, kind="ExternalInput")
    outT = nc.dram_tensor("out", (OUT_C, E_pad), BF16, kind="ExternalOutput")

    with tile.TileContext(nc) as tc:
        with (
            tc.tile_pool(name="const", bufs=1) as cpool,
            tc.tile_pool(name="idx", bufs=2) as ipool,
            tc.tile_pool(name="gat", bufs=2) as gpool,
            tc.tile_pool(name="attr", bufs=3) as apool,
            tc.tile_pool(name="pair", bufs=3) as ppool,
            tc.tile_pool(name="pairT", bufs=3) as ptpool,
            tc.tile_pool(name="hs", bufs=3) as hpool,
            tc.tile_pool(name="outp", bufs=3) as opool,
            tc.tile_pool(name="tp_ps", bufs=2, space="PSUM") as tp_ps,
            tc.tile_pool(name="h1_ps", bufs=2, space="PSUM") as h1_ps,
            tc.tile_pool(name="h2_ps", bufs=2, space="PSUM") as h2_ps,
            tc.tile_pool(name="o_ps", bufs=2, space="PSUM") as o_ps,
        ):
            ident = cpool.tile([128, 128], BF16)
            make_identity(nc, ident[:])
            w1 = cpool.tile([128, HIDDEN_C], BF16)
            nc.sync.dma_start(out=w1[:], in_=Wnnn.ap())
            w2 = cpool.tile([EDGE_C, HIDDEN_C], BF16)
            nc.sync.dma_start(out=w2[:], in_=Wroot.ap())
            w3 = cpool.tile([128, OUT_C], BF16)
            nc.sync.dma_start(out=w3[:], in_=Wout.ap())
            b1 = cpool.tile([HIDDEN_C, 1], F32)
            nc.sync.dma_start(out=b1[:], in_=bnnn.ap())
            b2 = cpool.tile([HIDDEN_C, 1], F32)
            nc.sync.dma_start(out=b2[:], in_=broot.ap())
            b3 = cpool.tile([OUT_C, 1], F32)
            nc.sync.dma_start(out=b3[:], in_=bout.ap())

            def issue_gathers(b):
                sb, db = b // N_BANKS, b % N_BANKS
                sidx_sb = ipool.tile([128, S], I16, tag="sidx")
                nc.sync.dma_start(out=sidx_sb[:], in_=sidx.ap()[b])
                didx_sb = ipool.tile([128, S], I16, tag="didx")
                nc.sync.dma_start(out=didx_sb[:], in_=didx.ap()[b])
                sgat = gpool.tile([128, (cap // 128) * 128], BF16, tag="sgat")
                nc.gpsimd.dma_gather(
                    out_ap=sgat[:].rearrange("p (t f) -> p t f", f=128),
                    in_ap=xsrc.ap()[sb * BANK:(sb + 1) * BANK, :],
                    idxs_ap=sidx_sb[:],
                    num_idxs=cap, num_idxs_reg=cap, elem_size=128,
                    single_packet=False, queue_num=(2 * b) % n_queues,
                )
                dgat = gpool.tile([128, (cap // 128) * 128], BF16, tag="dgat")
                nc.gpsimd.dma_gather(
                    out_ap=dgat[:].rearrange("p (t f) -> p t f", f=128),
                    in_ap=xdst.ap()[db * BANK:(db + 1) * BANK, :],
                    idxs_ap=didx_sb[:],
                    num_idxs=cap, num_idxs_reg=cap, elem_size=128,
                    single_packet=False, queue_num=(2 * b + 1) % n_queues,
                )
                return sgat, dgat

            from contextlib import nullcontext
            rep_cm = tc.For_i(0, repeat) if repeat > 1 else nullcontext()
            with rep_cm:
                gat_next = issue_gathers(0)
                for b in range(N_BUCKETS):
                    base = b * cap
                    srcG, dstG = gat_next
                    if b + 1 < N_BUCKETS:
                        gat_next = issue_gathers(b + 1)

                    for im in range(n_mega):
                        col = im * MEGA
                        attr_sb = apool.tile([EDGE_C, MEGA], BF16, tag="attr")
                        nc.sync.dma_start(
                            out=attr_sb[:],
                            in_=attrt.ap()[b][:, col:col + MEGA])
                        # pair rows [128e, 4, 128ch] = gathered src + dst chunks
                        pairG = ppool.tile([128, 4 * 128], BF16, tag="pair")
                        nc.vector.tensor_add(
                            out=pairG[:],
                            in0=srcG[:, col:col + MEGA],
                            in1=dstG[:, col:col + MEGA])
                        # PE-transpose each [128e, 128ch] block to [128ch, 128e]
                        pairT = ptpool.tile([128, MEGA], BF16, tag="pairT")
                        for j in range(4):
                            pj = tp_ps.tile([128, 128], BF16, tag="tp",
                                            space="PSUM")
                            nc.tensor.transpose(
                                out=pj[:], in_=pairG[:, j * 128:(j + 1) * 128],
                                identity=ident[:])
                            nc.vector.tensor_copy(
                                pairT[:, j * 128:(j + 1) * 128], pj[:])
                        h1 = h1_ps.tile([128, MEGA], F32, tag="h1", space="PSUM")
                        nc.tensor.matmul(out=h1[:], lhsT=w1[:], rhs=pairT[:],
                                         start=True, stop=True)
                        h2 = h2_ps.tile([128, MEGA], F32, tag="h2", space="PSUM")
                        nc.tensor.matmul(out=h2[:], lhsT=w2[:], rhs=attr_sb[:],
                                         start=True, stop=True)
                        hs1 = hpool.tile([128, MEGA], BF16, tag="hs1")
                        nc.scalar.activation(hs1[:], h1[:],
                                             mybir.ActivationFunctionType.Relu,
                                             bias=b1[:])
                        hs2 = hpool.tile([128, MEGA], BF16, tag="hs2")
                        nc.vector.tensor_scalar(
                            hs2[:], h2[:], b2[:], 0.0,
                            mybir.AluOpType.add, mybir.AluOpType.max)
                        hsum = hpool.tile([128, MEGA], BF16, tag="hsum")
                        nc.vector.tensor_add(out=hsum[:], in0=hs1[:], in1=hs2[:])
                        o = o_ps.tile([128, MEGA], F32, tag="o", space="PSUM")
                        nc.tensor.matmul(out=o[:], lhsT=w3[:], rhs=hsum[:],
                                         start=True, stop=True)
                        oT = opool.tile([128, MEGA], BF16, tag="oT")
                        nc.scalar.activation(oT[:], o[:],
                                             mybir.ActivationFunctionType.Relu,
                                             bias=b3[:])
                        nc.scalar.dma_start(
                            out=outT.ap()[:, base + col:base + col + MEGA],
                            in_=oT[:])
    nc.compile()
    _BUILD_CACHE[key] = nc
    return nc


def _host_prep(src_all, dst_all, edge_attr, cap, n_cores=N_CORES):
    E = src_all.shape[0]
    Ec = E // n_cores
    per_core = []
    for c in range(n_cores):
        lo, hi = c * Ec, (c + 1) * Ec
        src, dst = src_all[lo:hi], dst_all[lo:hi]
        bucket = (src // BANK) * N_BANKS + (dst // BANK)
        order = np.argsort(bucket, kind="stable")
        counts = np.bincount(bucket, minlength=N_BUCKETS)
        if counts.max() > cap:
            return None, int(counts.max())
        sorted_bucket = bucket[order]
        within = np.arange(Ec) - np.concatenate(([0], np.cumsum(counts)))[sorted_bucket]
        pos = sorted_bucket * cap + within
        sloc = np.zeros(N_BUCKETS * cap, np.int16)
        dloc = np.zeros(N_BUCKETS * cap, np.int16)
        sloc[pos] = (src[order] % BANK).astype(np.int16)
        dloc[pos] = (dst[order] % BANK).astype(np.int16)
        S = cap // 16

        def wrap(a):
            w = a.reshape(N_BUCKETS, S, 16).transpose(0, 2, 1)
            return np.ascontiguousarray(np.tile(w, (1, 8, 1)))

        attr_p = np.zeros((N_BUCKETS * cap, EDGE_C), BF16_NP)
        attr_p[pos] = edge_attr[lo:hi][order].astype(BF16_NP)
        # pre-transpose per bucket: [N_BUCKETS, 64ch, cap]
        attr_t = np.ascontiguousarray(
            attr_p.reshape(N_BUCKETS, cap, EDGE_C).transpose(0, 2, 1))
        per_core.append(dict(sidx=wrap(sloc), didx=wrap(dloc), attrt=attr_t,
                             meta=(order, pos)))
    return per_core, None


def kernel(x, edge_index, edge_attr, W_nnn, b_nnn, W_root, b_root, W_out, b_out,
           _repeat=1, _n_runs=1):
    x = np.asarray(x, np.float32)
    edge_index = np.asarray(edge_index)
    edge_attr = np.asarray(edge_attr, np.float32)
    W_nnn = np.asarray(W_nnn, np.float32).astype(BF16_NP)
    W_root = np.asarray(W_root, np.float32).astype(BF16_NP)
    W_out = np.asarray(W_out, np.float32).astype(BF16_NP)
    b_nnn = np.asarray(b_nnn, np.float32).reshape(-1, 1)
    b_root = np.asarray(b_root, np.float32).reshape(-1, 1)
    b_out = np.asarray(b_out, np.float32).reshape(-1, 1)
    E = edge_index.shape[1]
    src_all = np.ascontiguousarray(edge_index[0]).astype(np.int64)
    dst_all = np.ascontiguousarray(edge_index[1]).astype(np.int64)

    # padded bf16 node tables: xsrc = [x | 0], xdst = [0 | x]
    xb = x.astype(BF16_NP)
    xsrc_t = np.zeros((N_NODES, 128), BF16_NP)
    xsrc_t[:, :NODE_C] = xb
    xdst_t = np.zeros((N_NODES, 128), BF16_NP)
    xdst_t[:, NODE_C:] = xb

    cap = DEFAULT_CAP
    while True:
        per_core, max_count = _host_prep(src_all, dst_all, edge_attr, cap)
        if per_core is not None:
            break
        cap = ((max_count + MEGA - 1) // MEGA) * MEGA  # rare: grow and retry

    nc = _build_kernel(cap, repeat=_repeat)
    common = {"xsrc": xsrc_t, "xdst": xdst_t,
              "Wnnn": W_nnn, "Wroot": W_root, "Wout": W_out,
              "bnnn": b_nnn, "broot": b_root, "bout": b_out}
    in_maps = [{**common, "sidx": p["sidx"], "didx": p["didx"],
                "attrt": p["attrt"]} for p in per_core]
    res = None
    times = []
    for _ in range(max(1, _n_runs)):
        import time as _time
        t0 = _time.perf_counter()
        res = bass_utils.run_bass_kernel_spmd(nc, in_maps,
                                              core_ids=list(range(N_CORES)))
        times.append(_time.perf_counter() - t0)
    kernel.last_wall_times = times

    Ec = E // N_CORES
    E_pad = N_BUCKETS * cap
    full = np.empty((E, OUT_C), np.float32)
    for c in range(N_CORES):
        order, pos = per_core[c]["meta"]
        ot = res.results[c]["out"]  # [128, E_pad] bf16
        # blocked transpose to [E_pad, 128] f32 (cache-friendly)
        et = np.empty((E_pad, OUT_C), np.float32)
        B = 8192
        for e0 in range(0, E_pad, B):
            et[e0:e0 + B] = ot[:, e0:e0 + B].T
        full[c * Ec + order] = et[pos]
    return full


# revision 5
# speedup vs baseline: 1.2606x; 1.2606x over previous
"""Trainium2 Bass kernel for the GNN edge-MLP (nn_BMA_update):

    out[e] = relu( relu([x[src]|x[dst]] @ W_nnn + b_nnn)
                 + relu(edge_attr @ W_root + b_root) ) @ W_out -> relu

Strategy (edge-parallel across 8 NeuronCores, bf16 data movement):
  - Host shards edges into 8 contiguous slices; within each slice, edges are
    bucketed by (src_bank, dst_bank) over 4 banks of 25000 node rows so the
    GPSIMD dma_gather ucode (int16 indices) can fetch node rows; buckets are
    padded to a fixed per-bucket capacity so the device program is
    input-independent. The host un-permutes the per-edge output at the end.
  - Node features are staged in two bf16 tables padded to 128 channels:
    xsrc[i] = [x[i] | 0], xdst[i] = [0 | x[i]].  Each bucket's rows are
    fetched by four non-transpose dma_gathers (src/dst x two halves) spread
    over all 4 SWDGE queues, prefetched two buckets ahead.
  - Pair assembly: per 128-edge chunk, two accumulating PE matmuls against a
    bf16 identity transpose src and dst chunks into one f32 PSUM tile
    (pairT = srcT + dstT, exact thanks to the zero padding); a DVE copy drops
    it to bf16 SBUF.  No DVE op reads two SBUF operands, so GPSIMD's SWDGE
    descriptor generation never blocks on the shared SBUF port.
  - edge_attr is pre-transposed on the host per bucket ([64ch, cap] bf16), so
    h2's matmul rhs loads directly with plain contiguous DMA.
  - Per 512-edge megatile: h1/h2 matmuls (PSUM f32), ACT does relu(h1+b1) and
    half of the output relu, DVE does relu(h2+b2) and the other half; the
    W_out matmul accumulates W3^T hs1 + W3^T hs2 in PSUM (= W3^T(hs1+hs2)),
    eliminating the elementwise sum.  The output stays transposed
    ([128 out_ch, E_pad] bf16 in DRAM); the host transposes back to [E, 128]
    f32 (blocked, cheap) during un-permutation.
"""
import numpy as np
import ml_dtypes

import concourse.bacc as bacc
import concourse.mybir as mybir
import concourse.tile as tile
from concourse import bass_utils
from concourse.masks import make_identity

N_NODES = 100000
N_EDGES = 1600000
NODE_C = 64
EDGE_C = 64
HIDDEN_C = 128
OUT_C = 128
N_CORES = 8
N_BANKS = 4
BANK = N_NODES // N_BANKS          # 25000 (< 32768 so int16 indices work)
N_BUCKETS = N_BANKS * N_BANKS      # 16
MEGA = 512                         # edges per megatile
DEFAULT_CAP = 14336                # 28 megatiles; mean bucket load 12500
F32, BF16, I16 = mybir.dt.float32, mybir.dt.bfloat16, mybir.dt.int16
BF16_NP = ml_dtypes.bfloat16

_BUILD_CACHE = {}


def _build_kernel(cap, n_queues=4, repeat=1):
    key = (cap, n_queues, repeat)
    if key in _BUILD_CACHE:
        return _BUILD_CACHE[key]
    n_mega = cap // MEGA
    E_pad = N_BUCKETS * cap
    S = cap // 16
    H = cap // 2                     # gather split point (multiple of 512)

    nc = bacc.Bacc("TRN2", target_bir_lowering=False, debug=False,
                   num_swdge_queues=n_queues)
    xsrc = nc.dram_tensor("xsrc", (N_NODES, 128), BF16, kind="ExternalInput")
    xdst = nc.dram_tensor("xdst", (N_NODES, 128), BF16, kind="ExternalInput")
    sidx = nc.dram_tensor("sidx", (N_BUCKETS, 128, S), I16, kind="ExternalInput")
    didx = nc.dram_tensor("didx", (N_BUCKETS, 128, S), I16, kind="ExternalInput")
    attrt = nc.dram_tensor("attrt", (N_BUCKETS, EDGE_C, cap), BF16,
                           kind="ExternalInput")
    Wnnn = nc.dram_tensor("Wnnn", (2 * NODE_C, HIDDEN_C), BF16, kind="ExternalInput")
    Wroot = nc.dram_tensor("Wroot", (EDGE_C, HIDDEN_C), BF16, kind="ExternalInput")
    Wout = nc.dram_tensor("Wout", (HIDDEN_C, OUT_C), BF16, kind="ExternalInput")
    bnnn = nc.dram_tensor("bnnn", (HIDDEN_C, 1), F32, kind="ExternalInput")
    broot = nc.dram_tensor("broot", (HIDDEN_C, 1), F32, kind="ExternalInput")
    bout = nc.dram_tensor("bout", (OUT_C, 1), F32, kind="ExternalInput")
    outT = nc.dram_tensor("out", (OUT_C, E_pad), BF16, kind="ExternalOutput")

    with tile.TileContext(nc) as tc:
        with (
            tc.tile_pool(name="const", bufs=1) as cpool,
            tc.tile_pool(name="idx", bufs=3) as ipool,
            tc.tile_pool(name="gat", bufs=3) as gpool,
            tc.tile_pool(name="attr", bufs=3) as apool,
            tc.tile_pool(name="pairT", bufs=3) as ptpool,
            tc.tile_pool(name="hs", bufs=3) as hpool,
            tc.tile_pool(name="outp", bufs=3) as opool,
            tc.tile_pool(name="tp_ps", bufs=2, space="PSUM") as tp_ps,
            tc.tile_pool(name="h1_ps", bufs=2, space="PSUM") as h1_ps,
            tc.tile_pool(name="h2_ps", bufs=2, space="PSUM") as h2_ps,
            tc.tile_pool(name="o_ps", bufs=2, space="PSUM") as o_ps,
        ):
            ident = cpool.tile([128, 128], BF16)
            make_identity(nc, ident[:])
            w1 = cpool.tile([128, HIDDEN_C], BF16)
            nc.sync.dma_start(out=w1[:], in_=Wnnn.ap())
            w2 = cpool.tile([EDGE_C, HIDDEN_C], BF16)
            nc.sync.dma_start(out=w2[:], in_=Wroot.ap())
            w3 = cpool.tile([128, OUT_C], BF16)
            nc.sync.dma_start(out=w3[:], in_=Wout.ap())
            b1 = cpool.tile([HIDDEN_C, 1], F32)
            nc.sync.dma_start(out=b1[:], in_=bnnn.ap())
            b2 = cpool.tile([HIDDEN_C, 1], F32)
            nc.sync.dma_start(out=b2[:], in_=broot.ap())
            b3 = cpool.tile([OUT_C, 1], F32)
            nc.sync.dma_start(out=b3[:], in_=bout.ap())

            def issue_gathers(b):
                sb, db = b // N_BANKS, b % N_BANKS
                sidx_sb = ipool.tile([128, S], I16, tag="sidx")
                nc.sync.dma_start(out=sidx_sb[:], in_=sidx.ap()[b])
                didx_sb = ipool.tile([128, S], I16, tag="didx")
                nc.sync.dma_start(out=didx_sb[:], in_=didx.ap()[b])
                sgat = gpool.tile([128, cap], BF16, tag="sgat")
                dgat = gpool.tile([128, cap], BF16, tag="dgat")
                for (tbl, bank, it, gt, q) in (
                        (xsrc, sb, sidx_sb, sgat, 0),
                        (xsrc, sb, sidx_sb, sgat, 1),
                        (xdst, db, didx_sb, dgat, 2),
                        (xdst, db, didx_sb, dgat, 3)):
                    lo = 0 if q % 2 == 0 else H
                    nc.gpsimd.dma_gather(
                        out_ap=gt[:, lo:lo + H].rearrange("p (t f) -> p t f",
                                                          f=128),
                        in_ap=tbl.ap()[bank * BANK:(bank + 1) * BANK, :],
                        idxs_ap=it[:, lo // 16:(lo + H) // 16],
                        num_idxs=H, num_idxs_reg=H, elem_size=128,
                        single_packet=False, queue_num=q % n_queues,
                    )
                return sgat, dgat

            from contextlib import nullcontext
            rep_cm = tc.For_i(0, repeat) if repeat > 1 else nullcontext()
            with rep_cm:
                pending = [issue_gathers(0), issue_gathers(1)]
                for b in range(N_BUCKETS):
                    base = b * cap
                    srcG, dstG = pending.pop(0)
                    if b + 2 < N_BUCKETS:
                        pending.append(issue_gathers(b + 2))

                    for im in range(n_mega):
                        col = im * MEGA
                        attr_sb = apool.tile([EDGE_C, MEGA], BF16, tag="attr")
                        nc.sync.dma_start(
                            out=attr_sb[:],
                            in_=attrt.ap()[b][:, col:col + MEGA])
                        # pairT chunks: accumulate srcC^T + dstC^T in PSUM
                        pairT = ptpool.tile([128, MEGA], BF16, tag="pairT")
                        for j in range(4):
                            c0 = col + j * 128
                            pj = tp_ps.tile([128, 128], F32, tag="tp",
                                            space="PSUM")
                            nc.tensor.matmul(out=pj[:],
                                             lhsT=srcG[:, c0:c0 + 128],
                                             rhs=ident[:],
                                             start=True, stop=False)
                            nc.tensor.matmul(out=pj[:],
                                             lhsT=dstG[:, c0:c0 + 128],
                                             rhs=ident[:],
                                             start=False, stop=True)
                            nc.vector.tensor_copy(
                                pairT[:, j * 128:(j + 1) * 128], pj[:])
                        h1 = h1_ps.tile([128, MEGA], F32, tag="h1", space="PSUM")
                        nc.tensor.matmul(out=h1[:], lhsT=w1[:], rhs=pairT[:],
                                         start=True, stop=True)
                        h2 = h2_ps.tile([128, MEGA], F32, tag="h2", space="PSUM")
                        nc.tensor.matmul(out=h2[:], lhsT=w2[:], rhs=attr_sb[:],
                                         start=True, stop=True)
                        hs1 = hpool.tile([128, MEGA], BF16, tag="hs1")
                        nc.scalar.activation(hs1[:], h1[:],
                                             mybir.ActivationFunctionType.Relu,
                                             bias=b1[:])
                        hs2 = hpool.tile([128, MEGA], BF16, tag="hs2")
                        nc.vector.tensor_scalar(
                            hs2[:], h2[:], b2[:], 0.0,
                            mybir.AluOpType.add, mybir.AluOpType.max)
                        # o = W3^T hs1 + W3^T hs2  (= W3^T (hs1+hs2))
                        o = o_ps.tile([128, MEGA], F32, tag="o", space="PSUM")
                        nc.tensor.matmul(out=o[:], lhsT=w3[:], rhs=hs1[:],
                                         start=True, stop=False)
                        nc.tensor.matmul(out=o[:], lhsT=w3[:], rhs=hs2[:],
                                         start=False, stop=True)
                        oT = opool.tile([128, MEGA], BF16, tag="oT")
                        nc.scalar.activation(oT[:, 0:MEGA // 2],
                                             o[:, 0:MEGA // 2],
                                             mybir.ActivationFunctionType.Relu,
                                             bias=b3[:])
                        nc.vector.tensor_scalar(
                            oT[:, MEGA // 2:MEGA], o[:, MEGA // 2:MEGA],
                            b3[:], 0.0,
                            mybir.AluOpType.add, mybir.AluOpType.max)
                        nc.scalar.dma_start(
                            out=outT.ap()[:, base + col:base + col + MEGA],
                            in_=oT[:])
    nc.compile()
    _BUILD_CACHE[key] = nc
    return nc


def _host_prep(src_all, dst_all, edge_attr, cap, n_cores=N_CORES):
    E = src_all.shape[0]
    Ec = E // n_cores
    per_core = []
    for c in range(n_cores):
        lo, hi = c * Ec, (c + 1) * Ec
        src, dst = src_all[lo:hi], dst_all[lo:hi]
        bucket = (src // BANK) * N_BANKS + (dst // BANK)
        order = np.argsort(bucket, kind="stable")
        counts = np.bincount(bucket, minlength=N_BUCKETS)
        if counts.max() > cap:
            return None, int(counts.max())
        sorted_bucket = bucket[order]
        within = np.arange(Ec) - np.concatenate(([0], np.cumsum(counts)))[sorted_bucket]
        pos = sorted_bucket * cap + within
        sloc = np.zeros(N_BUCKETS * cap, np.int16)
        dloc = np.zeros(N_BUCKETS * cap, np.int16)
        sloc[pos] = (src[order] % BANK).astype(np.int16)
        dloc[pos] = (dst[order] % BANK).astype(np.int16)
        S = cap // 16

        def wrap(a):
            w = a.reshape(N_BUCKETS, S, 16).transpose(0, 2, 1)
            return np.ascontiguousarray(np.tile(w, (1, 8, 1)))

        attr_p = np.zeros((N_BUCKETS * cap, EDGE_C), BF16_NP)
        attr_p[pos] = edge_attr[lo:hi][order].astype(BF16_NP)
        # pre-transpose per bucket: [N_BUCKETS, 64ch, cap]
        attr_t = np.ascontiguousarray(
            attr_p.reshape(N_BUCKETS, cap, EDGE_C).transpose(0, 2, 1))
        per_core.append(dict(sidx=wrap(sloc), didx=wrap(dloc), attrt=attr_t,
                             meta=(order, pos)))
    return per_core, None


def kernel(x, edge_index, edge_attr, W_nnn, b_nnn, W_root, b_root, W_out, b_out,
           _repeat=1, _n_runs=1):
    x = np.asarray(x, np.float32)
    edge_index = np.asarray(edge_index)
    edge_attr = np.asarray(edge_attr, np.float32)
    W_nnn = np.asarray(W_nnn, np.float32).astype(BF16_NP)
    W_root = np.asarray(W_root, np.float32).astype(BF16_NP)
    W_out = np.asarray(W_out, np.float32).astype(BF16_NP)
    b_nnn = np.asarray(b_nnn, np.float32).reshape(-1, 1)
    b_root = np.asarray(b_root, np.float32).reshape(-1, 1)
    b_out = np.asarray(b_out, np.float32).reshape(-1, 1)
    E = edge_index.shape[1]
    src_all = np.ascontiguousarray(edge_index[0]).astype(np.int64)
    dst_all = np.ascontiguousarray(edge_index[1]).astype(np.int64)

    # padded bf16 node tables: xsrc = [x | 0], xdst = [0 | x]
    xb = x.astype(BF16_NP)
    xsrc_t = np.zeros((N_NODES, 128), BF16_NP)
    xsrc_t[:, :NODE_C] = xb
    xdst_t = np.zeros((N_NODES, 128), BF16_NP)
    xdst_t[:, NODE_C:] = xb

    cap = DEFAULT_CAP
    while True:
        per_core, max_count = _host_prep(src_all, dst_all, edge_attr, cap)
        if per_core is not None:
            break
        cap = ((max_count + MEGA - 1) // MEGA) * MEGA  # rare: grow and retry

    nc = _build_kernel(cap, repeat=_repeat)
    common = {"xsrc": xsrc_t, "xdst": xdst_t,
              "Wnnn": W_nnn, "Wroot": W_root, "Wout": W_out,
              "bnnn": b_nnn, "broot": b_root, "bout": b_out}
    in_maps = [{**common, "sidx": p["sidx"], "didx": p["didx"],
                "attrt": p["attrt"]} for p in per_core]
    res = None
    times = []
    for _ in range(max(1, _n_runs)):
        import time as _time
        t0 = _time.perf_counter()
        res = bass_utils.run_bass_kernel_spmd(nc, in_maps,
                                              core_ids=list(range(N_CORES)))
        times.append(_time.perf_counter() - t0)
    kernel.last_wall_times = times

    Ec = E // N_CORES
    E_pad = N_BUCKETS * cap
    full = np.empty((E, OUT_C), np.float32)
    for c in range(N_CORES):
        order, pos = per_core[c]["meta"]
        ot = res.results[c]["out"]  # [128, E_pad] bf16
        # blocked transpose to [E_pad, 128] f32 (cache-friendly)
        et = np.empty((E_pad, OUT_C), np.float32)
        B = 8192
        for e0 in range(0, E_pad, B):
            et[e0:e0 + B] = ot[:, e0:e0 + B].T
        full[c * Ec + order] = et[pos]
    return full


# revision 10
# speedup vs baseline: 9.8674x; 7.8274x over previous
"""Trainium2 Bass kernel for the GNN edge-MLP (nn_BMA_update):

    out[e] = relu( relu([x[src]|x[dst]] @ W_nnn + b_nnn)
                 + relu(edge_attr @ W_root + b_root) ) @ W_out -> relu

Strategy (edge-parallel across 8 NeuronCores, bf16 data movement):
  - Host shards edges into 8 contiguous slices; within each slice, edges are
    bucketed by (src_bank, dst_bank) over 4 banks of 25000 node rows so the
    GPSIMD dma_gather ucode (int16 indices) can fetch node rows; buckets are
    padded to a fixed per-bucket capacity so the device program is
    input-independent. The host un-permutes the per-edge output at the end.
  - Node features are staged in two bf16 tables padded to 128 channels:
    xsrc[i] = [x[i] | 0], xdst[i] = [0 | x[i]].  Each bucket's rows are
    fetched by four non-transpose dma_gathers (src/dst x two halves) spread
    over all 4 SWDGE queues, prefetched two buckets ahead.
  - Pair assembly: per 128-edge chunk, two accumulating PE matmuls against a
    bf16 identity transpose src and dst chunks into one f32 PSUM tile
    (pairT = srcT + dstT, exact thanks to the zero padding); a DVE copy drops
    it to bf16 SBUF.  No DVE op reads two SBUF operands, so GPSIMD's SWDGE
    descriptor generation never blocks on the shared SBUF port.
  - edge_attr is pre-transposed on the host per bucket ([64ch, cap] bf16), so
    h2's matmul rhs loads directly with plain contiguous DMA.
  - Per 512-edge megatile: h1/h2 matmuls (PSUM f32), ACT does relu(h1+b1) and
    half of the output relu, DVE does relu(h2+b2) and the other half; the
    W_out matmul accumulates W3^T hs1 + W3^T hs2 in PSUM (= W3^T(hs1+hs2)),
    eliminating the elementwise sum.  The output stays transposed
    ([128 out_ch, E_pad] bf16 in DRAM); the host transposes back to [E, 128]
    f32 (blocked, cheap) during un-permutation.
"""
import numpy as np
import ml_dtypes

import concourse.bacc as bacc
import concourse.mybir as mybir
import concourse.tile as tile
from concourse import bass_utils
from concourse.masks import make_identity

N_NODES = 100000
N_EDGES = 1600000
NODE_C = 64
EDGE_C = 64
HIDDEN_C = 128
OUT_C = 128
N_CORES = 8
N_BANKS = 4
BANK = N_NODES // N_BANKS          # 25000 (< 32768 so int16 indices work)
N_BUCKETS = N_BANKS * N_BANKS      # 16
MEGA = 512                         # edges per megatile
DEFAULT_CAP = 14336                # 28 megatiles; mean bucket load 12500
F32, BF16, I16 = mybir.dt.float32, mybir.dt.bfloat16, mybir.dt.int16
BF16_NP = ml_dtypes.bfloat16

_BUILD_CACHE = {}


def _build_kernel(cap, n_queues=4, repeat=1):
    key = (cap, n_queues, repeat)
    if key in _BUILD_CACHE:
        return _BUILD_CACHE[key]
    n_mega = cap // MEGA
    E_pad = N_BUCKETS * cap
    S = cap // 16
    H = cap // 2                     # gather split point (multiple of 512)

    nc = bacc.Bacc("TRN2", target_bir_lowering=False, debug=False,
                   num_swdge_queues=n_queues)
    xsrc = nc.dram_tensor("xsrc", (N_NODES, 128), BF16, kind="ExternalInput")
    xdst = nc.dram_tensor("xdst", (N_NODES, 128), BF16, kind="ExternalInput")
    sidx = nc.dram_tensor("sidx", (N_BUCKETS, 128, S), I16, kind="ExternalInput")
    didx = nc.dram_tensor("didx", (N_BUCKETS, 128, S), I16, kind="ExternalInput")
    attrt = nc.dram_tensor("attrt", (N_BUCKETS, EDGE_C, cap), BF16,
                           kind="ExternalInput")
    Wnnn = nc.dram_tensor("Wnnn", (2 * NODE_C, HIDDEN_C), BF16, kind="ExternalInput")
    Wroot = nc.dram_tensor("Wroot", (EDGE_C, HIDDEN_C), BF16, kind="ExternalInput")
    Wout = nc.dram_tensor("Wout", (HIDDEN_C, OUT_C), BF16, kind="ExternalInput")
    bnnn = nc.dram_tensor("bnnn", (HIDDEN_C, 1), F32, kind="ExternalInput")
    broot = nc.dram_tensor("broot", (HIDDEN_C, 1), F32, kind="ExternalInput")
    bout = nc.dram_tensor("bout", (OUT_C, 1), F32, kind="ExternalInput")
    outT = nc.dram_tensor("out", (OUT_C, E_pad), BF16, kind="ExternalOutput")

    with tile.TileContext(nc) as tc:
        with (
            tc.tile_pool(name="const", bufs=1) as cpool,
            tc.tile_pool(name="idx", bufs=3) as ipool,
            tc.tile_pool(name="gat", bufs=3) as gpool,
            tc.tile_pool(name="attr", bufs=2) as apool,
            tc.tile_pool(name="pairT", bufs=3) as ptpool,
            tc.tile_pool(name="hs", bufs=3) as hpool,
            tc.tile_pool(name="outp", bufs=2) as opool,
            tc.tile_pool(name="tp_ps", bufs=2, space="PSUM") as tp_ps,
            tc.tile_pool(name="h1_ps", bufs=2, space="PSUM") as h1_ps,
            tc.tile_pool(name="h2_ps", bufs=2, space="PSUM") as h2_ps,
            tc.tile_pool(name="o_ps", bufs=2, space="PSUM") as o_ps,
        ):
            ident = cpool.tile([128, 128], BF16)
            make_identity(nc, ident[:])
            w1 = cpool.tile([128, HIDDEN_C], BF16)
            nc.sync.dma_start(out=w1[:], in_=Wnnn.ap())
            w2 = cpool.tile([EDGE_C, HIDDEN_C], BF16)
            nc.sync.dma_start(out=w2[:], in_=Wroot.ap())
            w3 = cpool.tile([128, OUT_C], BF16)
            nc.sync.dma_start(out=w3[:], in_=Wout.ap())
            b1 = cpool.tile([HIDDEN_C, 1], F32)
            nc.sync.dma_start(out=b1[:], in_=bnnn.ap())
            b2 = cpool.tile([HIDDEN_C, 1], F32)
            nc.sync.dma_start(out=b2[:], in_=broot.ap())
            b3 = cpool.tile([OUT_C, 1], F32)
            nc.sync.dma_start(out=b3[:], in_=bout.ap())

            def issue_gathers(b, half):
                sb, db = b // N_BANKS, b % N_BANKS
                c0, c1 = half * (S // 2), (half + 1) * (S // 2)
                sidx_sb = ipool.tile([128, S // 2], I16, tag="sidx")
                nc.sync.dma_start(out=sidx_sb[:], in_=sidx.ap()[b][:, c0:c1])
                didx_sb = ipool.tile([128, S // 2], I16, tag="didx")
                nc.sync.dma_start(out=didx_sb[:], in_=didx.ap()[b][:, c0:c1])
                sgat = gpool.tile([128, H], BF16, tag="sgat")
                dgat = gpool.tile([128, H], BF16, tag="dgat")
                Q = H // 2
                for (tbl, bank, it, gt, q) in (
                        (xsrc, sb, sidx_sb, sgat, 0),
                        (xsrc, sb, sidx_sb, sgat, 1),
                        (xdst, db, didx_sb, dgat, 2),
                        (xdst, db, didx_sb, dgat, 3)):
                    lo = 0 if q % 2 == 0 else Q
                    nc.gpsimd.dma_gather(
                        out_ap=gt[:, lo:lo + Q].rearrange("p (t f) -> p t f",
                                                          f=128),
                        in_ap=tbl.ap()[bank * BANK:(bank + 1) * BANK, :],
                        idxs_ap=it[:, lo // 16:(lo + Q) // 16],
                        num_idxs=Q, num_idxs_reg=Q, elem_size=128,
                        single_packet=False, queue_num=q % n_queues,
                    )
                return sgat, dgat

            from contextlib import nullcontext
            rep_cm = tc.For_i(0, repeat) if repeat > 1 else nullcontext()
            with rep_cm:
                units = [(b, h) for b in range(N_BUCKETS) for h in range(2)]
                LEAD = 3
                pending = [issue_gathers(*units[i]) for i in range(LEAD)]
                for ui, (b, half) in enumerate(units):
                    base = b * cap
                    srcG, dstG = pending.pop(0)
                    if ui + LEAD < len(units):
                        pending.append(issue_gathers(*units[ui + LEAD]))

                    if True:
                        hbase = half * H
                        # batched attr load + batched out store (half bucket)
                        attr_sb = apool.tile([EDGE_C, H], BF16, tag="attr")
                        nc.sync.dma_start(
                            out=attr_sb[:],
                            in_=attrt.ap()[b][:, hbase:hbase + H])
                        oT = opool.tile([128, H], BF16, tag="oT")
                        for im in range(H // MEGA):
                            ocol = im * MEGA
                            # pairT chunks: accumulate srcC^T + dstC^T in PSUM
                            pairT = ptpool.tile([128, MEGA], BF16, tag="pairT")
                            for j in range(4):
                                c0 = ocol + j * 128
                                pj = tp_ps.tile([128, 128], F32, tag="tp",
                                                space="PSUM")
                                nc.tensor.matmul(out=pj[:],
                                                 lhsT=srcG[:, c0:c0 + 128],
                                                 rhs=ident[:],
                                                 start=True, stop=False)
                                nc.tensor.matmul(out=pj[:],
                                                 lhsT=dstG[:, c0:c0 + 128],
                                                 rhs=ident[:],
                                                 start=False, stop=True)
                                nc.vector.tensor_copy(
                                    pairT[:, j * 128:(j + 1) * 128], pj[:])
                            h1 = h1_ps.tile([128, MEGA], F32, tag="h1",
                                            space="PSUM")
                            nc.tensor.matmul(out=h1[:], lhsT=w1[:],
                                             rhs=pairT[:],
                                             start=True, stop=True)
                            h2 = h2_ps.tile([128, MEGA], F32, tag="h2",
                                            space="PSUM")
                            nc.tensor.matmul(
                                out=h2[:], lhsT=w2[:],
                                rhs=attr_sb[:, ocol:ocol + MEGA],
                                start=True, stop=True)
                            hs1 = hpool.tile([128, MEGA], BF16, tag="hs1")
                            nc.scalar.activation(
                                hs1[:], h1[:],
                                mybir.ActivationFunctionType.Relu,
                                bias=b1[:])
                            hs2 = hpool.tile([128, MEGA], BF16, tag="hs2")
                            nc.vector.tensor_scalar(
                                hs2[:], h2[:], b2[:], 0.0,
                                mybir.AluOpType.add, mybir.AluOpType.max)
                            # o = W3^T hs1 + W3^T hs2  (= W3^T (hs1+hs2))
                            o = o_ps.tile([128, MEGA], F32, tag="o",
                                          space="PSUM")
                            nc.tensor.matmul(out=o[:], lhsT=w3[:], rhs=hs1[:],
                                             start=True, stop=False)
                            nc.tensor.matmul(out=o[:], lhsT=w3[:], rhs=hs2[:],
                                             start=False, stop=True)
                            nc.scalar.activation(
                                oT[:, ocol:ocol + MEGA // 2],
                                o[:, 0:MEGA // 2],
                                mybir.ActivationFunctionType.Relu,
                                bias=b3[:])
                            nc.vector.tensor_scalar(
                                oT[:, ocol + MEGA // 2:ocol + MEGA],
                                o[:, MEGA // 2:MEGA],
                                b3[:], 0.0,
                                mybir.AluOpType.add, mybir.AluOpType.max)
                        nc.scalar.dma_start(
                            out=outT.ap()[:, base + hbase:base + hbase + H],
                            in_=oT[:])
    nc.compile()
    _BUILD_CACHE[key] = nc
    return nc


def _host_prep(src_all, dst_all, edge_attr, cap, n_cores=N_CORES):
    E = src_all.shape[0]
    Ec = E // n_cores
    per_core = []
    for c in range(n_cores):
        lo, hi = c * Ec, (c + 1) * Ec
        src, dst = src_all[lo:hi], dst_all[lo:hi]
        bucket = (src // BANK) * N_BANKS + (dst // BANK)
        order = np.argsort(bucket, kind="stable")
        counts = np.bincount(bucket, minlength=N_BUCKETS)
        if counts.max() > cap:
            return None, int(counts.max())
        sorted_bucket = bucket[order]
        within = np.arange(Ec) - np.concatenate(([0], np.cumsum(counts)))[sorted_bucket]
        pos = sorted_bucket * cap + within
        sloc = np.zeros(N_BUCKETS * cap, np.int16)
        dloc = np.zeros(N_BUCKETS * cap, np.int16)
        sloc[pos] = (src[order] % BANK).astype(np.int16)
        dloc[pos] = (dst[order] % BANK).astype(np.int16)
        S = cap // 16

        def wrap(a):
            w = a.reshape(N_BUCKETS, S, 16).transpose(0, 2, 1)
            return np.ascontiguousarray(np.tile(w, (1, 8, 1)))

        attr_p = np.zeros((N_BUCKETS * cap, EDGE_C), BF16_NP)
        attr_p[pos] = edge_attr[lo:hi][order].astype(BF16_NP)
        # pre-transpose per bucket: [N_BUCKETS, 64ch, cap]
        attr_t = np.ascontiguousarray(
            attr_p.reshape(N_BUCKETS, cap, EDGE_C).transpose(0, 2, 1))
        per_core.append(dict(sidx=wrap(sloc), didx=wrap(dloc), attrt=attr_t,
                             meta=(order, pos)))
    return per_core, None


def kernel(x, edge_index, edge_attr, W_nnn, b_nnn, W_root, b_root, W_out, b_out,
           _repeat=1, _n_runs=1):
    x = np.asarray(x, np.float32)
    edge_index = np.asarray(edge_index)
    edge_attr = np.asarray(edge_attr, np.float32)
    W_nnn = np.asarray(W_nnn, np.float32).astype(BF16_NP)
    W_root = np.asarray(W_root, np.float32).astype(BF16_NP)
    W_out = np.asarray(W_out, np.float32).astype(BF16_NP)
    b_nnn = np.asarray(b_nnn, np.float32).reshape(-1, 1)
    b_root = np.asarray(b_root, np.float32).reshape(-1, 1)
    b_out = np.asarray(b_out, np.float32).reshape(-1, 1)
    E = edge_index.shape[1]
    src_all = np.ascontiguousarray(edge_index[0]).astype(np.int64)
    dst_all = np.ascontiguousarray(edge_index[1]).astype(np.int64)

    # padded bf16 node tables: xsrc = [x | 0], xdst = [0 | x]
    xb = x.astype(BF16_NP)
    xsrc_t = np.zeros((N_NODES, 128), BF16_NP)
    xsrc_t[:, :NODE_C] = xb
    xdst_t = np.zeros((N_NODES, 128), BF16_NP)
    xdst_t[:, NODE_C:] = xb

    cap = DEFAULT_CAP
    while True:
        per_core, max_count = _host_prep(src_all, dst_all, edge_attr, cap)
        if per_core is not None:
            break
        cap = ((max_count + MEGA - 1) // MEGA) * MEGA  # rare: grow and retry

    nc = _build_kernel(cap, repeat=_repeat)
    common = {"xsrc": xsrc_t, "xdst": xdst_t,
              "Wnnn": W_nnn, "Wroot": W_root, "Wout": W_out,
              "bnnn": b_nnn, "broot": b_root, "bout": b_out}
    in_maps = [{**common, "sidx": p["sidx"], "didx": p["didx"],
                "attrt": p["attrt"]} for p in per_core]
    res = None
    times = []
    for _ in range(max(1, _n_runs)):
        import time as _time
        t0 = _time.perf_counter()
        res = bass_utils.run_bass_kernel_spmd(nc, in_maps,
                                              core_ids=list(range(N_CORES)))
        times.append(_time.perf_counter() - t0)
    kernel.last_wall_times = times

    Ec = E // N_CORES
    E_pad = N_BUCKETS * cap
    full = np.empty((E, OUT_C), np.float32)
    for c in range(N_CORES):
        order, pos = per_core[c]["meta"]
        ot = res.results[c]["out"]  # [128, E_pad] bf16
        # blocked transpose to [E_pad, 128] f32 (cache-friendly)
        et = np.empty((E_pad, OUT_C), np.float32)
        B = 8192
        for e0 in range(0, E_pad, B):
            et[e0:e0 + B] = ot[:, e0:e0 + B].T
        full[c * Ec + order] = et[pos]
    return full
